# revision 33
# baseline (speedup 1.0000x reference)
"""Cross multi-headed attention with relative position bias, sharded over 8 trn2 cores.

Sharding: query positions (T1=1024) split 128/core. pos_k (the 256MB tensor) is
indexed by query position, so each core streams a disjoint 1/8 slice exactly
once. K/V are computed from the full x on every core (cheap); outputs are
disjoint q-slices gathered on host.

Layout tricks:
  - LayerNorm is folded on the host: xn^T ships pre-packed in bf16, so the
    device does no stats/affine/transpose work at all.
  - pos_k is host-packed to (qgroup-of-4, dk+B, 4*T2) so it streams as the
    matmul moving operand (contraction dim dk on partitions) in 590KB DMAs.
  - The mask penalty is folded into the Bmat matmul as 8 extra contraction
    rows (one-hot per batch x -1e4 penalty): masking costs zero vector ops and
    masked attn weights are exactly 0 after exp underflow.
  - Bmat comes out of its matmul in per-q tiles (32 bh-rows x T2); an
    SBUF->SBUF DMA row-scatter regroups it into a resident per-(b,h) tensor
    (128q x T2) -- no DRAM roundtrip.  An identity-matmul accumulates it
    straight onto the QK^T scores in PSUM.
  - Softmax denominators come from the exp activation's accum_out for free;
    no max-subtraction needed since scores are bounded (~N(0,2) after scale).
"""

import numpy as np
import ml_dtypes
from contextlib import ExitStack

import concourse.bass as bass
import concourse.tile as tile
import concourse.mybir as mybir
from concourse import bacc
from concourse.bass_utils import run_bass_kernel_spmd
from concourse.masks import make_identity

F32 = mybir.dt.float32
F32R = mybir.dt.float32r
BF16 = mybir.dt.bfloat16

B, T, F, H, DK = 8, 1024, 256, 4, 64
NCORES = 8
TQ = T // NCORES          # query rows per core (128)
TOK = B * T               # tokens for K/V (8192)
TOKQ = B * TQ             # query tokens per core (1024)
NBH = B * H               # 32 (b,h) pairs
KAUG = DK + B             # bmat contraction rows: 64 qk + 8 mask-penalty
GQ = 4                    # q rows per bmat PSUM group
NQU = TQ // GQ            # 32 bmat groups (4-q units)
PBQ = 4                   # q rows per posk DMA block (1 group)
NPB = TQ // PBQ           # 16 posk blocks
NWB = TQ // 16            # 8 bmat write blocks (16 q each)
EPS = 1e-5
PEN = 1e4                 # mask penalty (score -= PEN/8 => exp underflows to 0)

CFG = dict()

AF = mybir.ActivationFunctionType
ALU = mybir.AluOpType


def build_program(cfg=CFG, bv_nz=False, bo_nz=False):
    nc = bacc.Bacc("TRN2", target_bir_lowering=False, debug=False)

    # ---- DRAM I/O ----
    # xn^T in token-halves: [tokhalf, f//128, f%128, tok%4096]
    xnt_d = nc.dram_tensor("xnt", [2, 2, 128, TOK // 2], BF16, kind="ExternalInput")
    qt_d = nc.dram_tensor("q_t", [F, TOKQ], BF16, kind="ExternalInput")
    # pos_k packed 8 q per block: [pb, k, s*T + t] (16KB contiguous rows)
    posk_d = nc.dram_tensor("posk_aug", [NPB, KAUG, PBQ * T], BF16,
                            kind="ExternalInput")
    # permuted identity for the bmat accumulate: P[r, q] = 1 iff q = perm(r)
    idp_d = nc.dram_tensor("idperm", [128, 128], BF16, kind="ExternalInput")
    oneh_d = nc.dram_tensor("onehot", [B, TQ * NBH], BF16, kind="ExternalInput")
    wq_d = nc.dram_tensor("wq", [F, F], BF16, kind="ExternalInput")
    wk_d = nc.dram_tensor("wk2", [F, F], BF16, kind="ExternalInput")
    wv_d = nc.dram_tensor("wv2", [F, F], BF16, kind="ExternalInput")
    wo_d = nc.dram_tensor("wo", [F, F], F32R, kind="ExternalInput")
    bq_d = nc.dram_tensor("bq_cols", [128, 2], F32, kind="ExternalInput")
    bk_d = nc.dram_tensor("bk2_cols", [128, 2], F32, kind="ExternalInput")
    bv_d = nc.dram_tensor("bv2_row", [1, F], BF16, kind="ExternalInput")
    bo_d = nc.dram_tensor("bo_row", [1, F], F32R, kind="ExternalInput")
    ones_d = nc.dram_tensor("ones_row", [1, 128], F32R, kind="ExternalInput")
    # out[q, b*F + f]; host reshapes to (B, TQ, F)
    out_d = nc.dram_tensor("out", [TQ, B * F], F32, kind="ExternalOutput")

    with tile.TileContext(nc) as tc, ExitStack() as ctx:
        consts = ctx.enter_context(tc.tile_pool(name="consts", bufs=1))
        persist = ctx.enter_context(tc.tile_pool(name="persist", bufs=1))

        # ---- constants ----
        id_bf = consts.tile([128, 128], BF16)
        make_identity(nc, id_bf)
        idp_sb = consts.tile([128, 128], BF16)
        nc.gpsimd.dma_start(idp_sb[:], idp_d[:])
        wq_sb = consts.tile([128, 2 * F], BF16)   # [f%128, (f//128)*F + n]
        wk_sb = consts.tile([128, 2 * F], BF16)
        wv_sb = consts.tile([128, 2 * F], BF16)
        wo_sb = consts.tile([64, 4 * F], F32R)    # [hd%64, (hd//64)*F + n]
        for w_sb, w_d in ((wq_sb, wq_d), (wk_sb, wk_d), (wv_sb, wv_d)):
            for fc in range(2):
                nc.gpsimd.dma_start(w_sb[:, fc * F:(fc + 1) * F],
                                    w_d[fc * 128:(fc + 1) * 128, :])
        for ci in range(4):
            nc.gpsimd.dma_start(wo_sb[:, ci * F:(ci + 1) * F],
                                wo_d[ci * 64:(ci + 1) * 64, :])
        bq_sb = consts.tile([128, 2], F32)
        nc.gpsimd.dma_start(bq_sb[:], bq_d[:])
        bk_sb = consts.tile([128, 2], F32)
        nc.gpsimd.dma_start(bk_sb[:], bk_d[:])
        if bv_nz:
            bv_sb = consts.tile([1, F], BF16)
            nc.gpsimd.dma_start(bv_sb[:], bv_d[:])
            ones_bf = consts.tile([1, 128], BF16)
            nc.gpsimd.memset(ones_bf[:], 1.0)
        if bo_nz:
            bo_sb = consts.tile([1, F], F32R)
            nc.gpsimd.dma_start(bo_sb[:], bo_d[:])
            ones_r = consts.tile([1, 128], F32R)
            nc.gpsimd.dma_start(ones_r[:], ones_d[:])

        # ---- persistent activations ----
        kt_sb = persist.tile([128, 2 * TOK], BF16)     # K^T: [n%128, (n//128)*TOK+tok]
        v_sb = persist.tile([128, (TOK // 128) * F], BF16)  # V: [tok%128,(tok//128)*F+hd]
        qq_sb = persist.tile([KAUG, TQ * NBH], BF16)   # [d|b', b*H*TQ + h*TQ + q]
        qq_dup = persist.tile([128, TQ * NBH], BF16)   # rows 64:128 = qq_sb[0:64]
        nc.gpsimd.dma_start(qq_sb[DK:KAUG, :], oneh_d[:])
        dram = ctx.enter_context(tc.tile_pool(name="dram", bufs=1, space="DRAM"))
        # row r = g4*16 + j*4 + qgi holds q = g4*16 + qgi*4 + j (idperm undoes)
        bmat_dram = dram.tile([NBH, NWB, GQ, 4, T], BF16)

        # ================= Phase B: Q^T projection -> qq_sb =================
        with tc.tile_pool(name="qstage", bufs=1) as qstage, \
             tc.tile_pool(name="ps_q", bufs=2, space="PSUM") as ps_q:
            qt_tiles = []
            for fc in range(2):
                qt = qstage.tile([128, TOKQ], BF16, tag=f"qt{fc}")
                nc.sync.dma_start(qt[:], qt_d[fc * 128:(fc + 1) * 128, :])
                qt_tiles.append(qt)
            qv = qq_sb[0:DK].rearrange("p (b h q) -> p b q h", h=H, q=TQ)
            for h in range(H):
                qp = ps_q.tile([DK, TOKQ], F32, tag="qp")
                for half in range(2):
                    for fc in range(2):
                        nc.tensor.matmul(
                            qp[:, half * 512:(half + 1) * 512],
                            wq_sb[:, fc * F + h * DK: fc * F + (h + 1) * DK],
                            qt_tiles[fc][:, half * 512:(half + 1) * 512],
                            start=(fc == 0), stop=(fc == 1))
                src = qp.rearrange("p (b q) -> p b q", b=B)
                bias = bq_sb[64 * (h % 2):64 * (h % 2) + 64, h // 2: h // 2 + 1]
                nc.scalar.activation(qv[:, :, :, h], src, AF.Identity, bias=bias)
            nc.sync.dma_start(qq_dup[64:128, :], qq_sb[0:DK, :])

        # ===== Phases C+D interleaved: K^T/V proj, Bmat stream + regroup =====
        with tc.tile_pool(name="xstage", bufs=1) as xstage, \
             tc.tile_pool(name="poskst", bufs=6) as poskst, \
             tc.tile_pool(name="bmstage", bufs=2) as bmstage, \
             tc.tile_pool(name="ps_bm", bufs=2, space="PSUM") as ps_bm, \
             tc.tile_pool(name="ps_kv", bufs=3, space="PSUM") as ps_kv:

            xnt = xstage.tile([128, TOK], BF16, tag="xnt")  # [f%128, (f//128)*4096+tl]
            qq_v = qq_sb.rearrange("p (b h q) -> p b h q", h=H, q=TQ)

            def kv_chunk(s):
                # K^T and V for token chunk s (512 tokens)
                L, sl = s // 8, s % 8
                tok0 = s * 512
                for ncI in range(2):
                    kp = ps_kv.tile([128, 512], F32, tag="kv")
                    for fc in range(2):
                        nc.tensor.matmul(
                            kp[:],
                            wk_sb[:, fc * F + ncI * 128: fc * F + (ncI + 1) * 128],
                            xnt[:, fc * (TOK // 2) + sl * 512:
                                fc * (TOK // 2) + (sl + 1) * 512],
                            start=(fc == 0), stop=(fc == 1))
                    nc.vector.tensor_scalar(
                        out=kt_sb[:, ncI * TOK + tok0: ncI * TOK + tok0 + 512],
                        in0=kp[:], scalar1=bk_sb[:, ncI:ncI + 1], scalar2=None,
                        op0=ALU.add)
                for gp in range(2):  # 2 token-tiles per V psum
                    vp = ps_kv.tile([128, 2 * F], F32, tag="kv")
                    for gi in range(2):
                        g = sl * 4 + gp * 2 + gi
                        for fc in range(2):
                            nc.tensor.matmul(
                                vp[:, gi * F:(gi + 1) * F],
                                xnt[:, fc * (TOK // 2) + g * 128:
                                    fc * (TOK // 2) + (g + 1) * 128],
                                wv_sb[:, fc * F:(fc + 1) * F],
                                start=(fc == 0), stop=(fc == 1 and not bv_nz))
                        if bv_nz:
                            nc.tensor.matmul(vp[:, gi * F:(gi + 1) * F],
                                             ones_bf[:], bv_sb[:],
                                             start=False, stop=True)
                    g0 = s * 4 + gp * 2
                    nc.vector.tensor_copy(v_sb[:, g0 * F:(g0 + 2) * F], vp[:])

            pk = None
            bs_wide = None
            for qu in range(NQU):
                if qu % 16 == 0:
                    L = qu // 16
                    for fc in range(2):
                        nc.gpsimd.dma_start(
                            xnt[:, fc * (TOK // 2):(fc + 1) * (TOK // 2)],
                            xnt_d[L, fc])
                pk = poskst.tile([KAUG, PBQ * T], BF16, tag="pk")
                nc.sync.dma_start(pk[:], posk_d[qu])
                if qu % 4 == 0:
                    bs_wide = bmstage.tile([128, 4 * T], BF16, tag="bsw")
                bp = ps_bm.tile([128, T], F32, tag="bp")
                for j in range(GQ):
                    q = qu * GQ + j
                    lhs = qq_v[:, :, :, q]
                    pc = j * T
                    for half in range(2):
                        nc.tensor.matmul(
                            bp[32 * j:32 * (j + 1), half * 512:(half + 1) * 512],
                            lhs,
                            pk[:, pc + half * 512: pc + (half + 1) * 512],
                            start=True, stop=True, tile_position=(0, 32 * j))
                wsub = qu % 4
                nc.scalar.activation(bs_wide[:, wsub * T:(wsub + 1) * T],
                                     bp[:], AF.Copy)
                if wsub == 3:
                    # write block: row (j, bh) -> dram[bh, g4, j, qgi, t] (8KB)
                    dst = bmat_dram[:, qu // 4].rearrange(
                        "bh j qgi t -> j bh qgi t")
                    nc.scalar.dma_start(dst, bs_wide[:])
                if qu % 2 == 1:
                    kv_chunk(qu // 2)

        # ================= Phase E: attention per (b, h) =================
        with tc.tile_pool(name="attst", bufs=4) as attst, \
             tc.tile_pool(name="ps_s", bufs=2, space="PSUM") as ps_s, \
             tc.tile_pool(name="ps_at", bufs=2, space="PSUM") as ps_at, \
             tc.tile_pool(name="ps_small", bufs=1, space="PSUM") as ps_small, \
             tc.tile_pool(name="avout", bufs=2) as avout, \
             tc.tile_pool(name="obuf", bufs=1) as obuf:
            qa0 = qq_sb[0:DK].rearrange("p (c q) -> p c q", c=NBH)
            qa1 = qq_dup[64:128].rearrange("p (c q) -> p c q", c=NBH)
            ob_all = obuf.tile([128, B * F], F32)
            for b in range(B):
                avt = avout.tile([64, 4 * TQ], F32R, tag="avt")  # [hd%64,(hd//64)*TQ+q]
                for h in range(H):
                    bh = H * b + h
                    lhsq = (qa0 if h % 2 == 0 else qa1)[:, bh, :]
                    bm = attst.tile([128, T], BF16, tag="bm")
                    nc.sync.dma_start(bm[:], bmat_dram[bh])
                    sp = ps_s.tile([128, T], F32, tag="sp")
                    for half in range(2):
                        nc.tensor.matmul(
                            sp[:, half * 512:(half + 1) * 512], lhsq,
                            kt_sb[(h % 2) * 64:(h % 2) * 64 + 64,
                                  (h // 2) * TOK + b * T + half * 512:
                                  (h // 2) * TOK + b * T + (half + 1) * 512],
                            start=True, stop=False)
                        nc.tensor.matmul(
                            sp[:, half * 512:(half + 1) * 512], idp_sb[:],
                            bm[:, half * 512:(half + 1) * 512],
                            start=False, stop=True)
                    attn = attst.tile([128, T], BF16, tag="attn")
                    den = attst.tile([128, 1], F32, tag="den")
                    nc.scalar.activation(attn[:], sp[:], AF.Exp,
                                         scale=1.0 / np.sqrt(DK), accum_out=den[:])
                    rn = attst.tile([128, 1], F32, tag="rn")
                    nc.vector.reciprocal(rn[:], den[:])
                    # normalize in [q, t] layout (per-partition scale) on DVE
                    ats = attst.tile([128, T], BF16, tag="ats")
                    nc.vector.tensor_scalar(out=ats[:], in0=attn[:], scalar1=rn[:],
                                            scalar2=None, op0=ALU.mult)
                    att = attst.tile([128, T], BF16, tag="att")  # attn^T
                    tp2 = ps_at.tile([128, T], BF16, tag="tp2")
                    for c in range(8):
                        nc.tensor.transpose(tp2[:, c * 128:(c + 1) * 128],
                                            ats[:, c * 128:(c + 1) * 128], id_bf[:])
                    nc.vector.tensor_copy(att[:], tp2[:])
                    # AV with v stationary: out = av^T [d, q], feeds out-proj direct
                    avp = ps_small.tile([DK, TQ], F32, tag="avp")
                    for c in range(8):
                        nc.tensor.matmul(
                            avp[:],
                            v_sb[:, (b * 8 + c) * F + h * DK:
                                 (b * 8 + c) * F + (h + 1) * DK],
                            att[:, c * 128:(c + 1) * 128],
                            start=(c == 0), stop=(c == 7))
                    nc.vector.tensor_copy(avt[:, h * TQ:(h + 1) * TQ], avp[:])
                op = ps_small.tile([128, F], F32, tag="op")
                for ci in range(4):
                    nc.tensor.matmul(
                        op[:], avt[:, ci * TQ:(ci + 1) * TQ],
                        wo_sb[:, ci * F:(ci + 1) * F],
                        start=(ci == 0), stop=(ci == 3 and not bo_nz))
                if bo_nz:
                    nc.tensor.matmul(op[:], ones_r[:], bo_sb[:],
                                     start=False, stop=True)
                nc.vector.tensor_copy(ob_all[:, b * F:(b + 1) * F], op[:])
            nc.scalar.dma_start(out_d[:], ob_all[:])

    nc._dbg_names = {
        "kt": kt_sb.tensor.name, "v": v_sb.tensor.name,
        "qq": qq_sb.tensor.name, "bmat": bmat_dram.tensor.name,
    }
    nc.compile()
    return nc


def make_core_inputs(inputs, cfg=CFG):
    """Host-side sharding/layout. Returns (per_core_maps, bias_flags)."""
    x = np.asarray(inputs["x"], np.float32)
    q_in = np.asarray(inputs["q_in"], np.float32)
    pos_k = np.asarray(inputs["pos_k"], np.float32)
    mask = np.asarray(inputs["mask"])
    ln_g = np.asarray(inputs["ln_g"], np.float32)
    ln_b = np.asarray(inputs["ln_b"], np.float32)
    Wq, bq = np.asarray(inputs["Wq"], np.float32), np.asarray(inputs["bq"], np.float32)
    Wk, bk = np.asarray(inputs["Wk"], np.float32), np.asarray(inputs["bk"], np.float32)
    Wv, bv = np.asarray(inputs["Wv"], np.float32), np.asarray(inputs["bv"], np.float32)
    Wo, bo = np.asarray(inputs["Wo"], np.float32), np.asarray(inputs["bo"], np.float32)

    bf = ml_dtypes.bfloat16

    # LayerNorm on host (f32), then pack xn^T
    mu = x.mean(-1, keepdims=True)
    xc = x - mu
    var = (xc * xc).mean(-1, keepdims=True)
    xn = xc / np.sqrt(var + EPS) * ln_g + ln_b
    # xnt_d[L, fc, p, tl] = xn^T[fc*128+p, L*4096+tl]
    xnt = np.ascontiguousarray(
        xn.reshape(TOK, F).T.reshape(2, 128, 2, TOK // 2).transpose(2, 0, 1, 3)
    ).astype(bf)

    shared = {
        "xnt": xnt,
        "wq": Wq.astype(bf),
        "wk2": Wk.astype(bf),
        "wv2": Wv.astype(bf),
        "wo": Wo.astype(np.float32),
        "bq_cols": np.ascontiguousarray(bq.reshape(2, 128).T).astype(np.float32),
        "bk2_cols": np.ascontiguousarray(bk.reshape(2, 128).T).astype(np.float32),
        "bv2_row": bv.reshape(1, F).astype(bf),
        "bo_row": bo.reshape(1, F).astype(np.float32),
        "ones_row": np.ones((1, 128), np.float32),
    }
    # onehot penalty rows: qq_aug[DK+b', b*H*TQ + h*TQ + q] = -PEN iff b'==b
    oh = np.zeros((B, B, H * TQ), np.float32)
    for bb in range(B):
        oh[bb, bb, :] = -PEN
    shared["onehot"] = np.ascontiguousarray(oh.reshape(B, NBH * TQ)).astype(bf)
    # permuted identity: dram row r = g4*16 + j*4 + qgi holds q = g4*16 + qgi*4 + j
    idp = np.zeros((128, 128), np.float32)
    r = np.arange(128)
    qperm = (r // 16) * 16 + (r % 4) * 4 + (r // 4) % 4
    idp[r, qperm] = 1.0
    shared["idperm"] = idp.astype(bf)

    per_core = []
    for c in range(NCORES):
        qs = slice(c * TQ, (c + 1) * TQ)
        pa = np.empty((TQ, KAUG, T), bf)
        pa[:, :DK, :] = pos_k[qs].transpose(0, 2, 1).astype(bf)
        pa[:, DK:, :] = (1.0 - mask[:, qs, :].astype(np.float32)
                         ).transpose(1, 0, 2).astype(bf)
        # pack PBQ q per block: [pb, k, s*T + t]
        pa = np.ascontiguousarray(
            pa.reshape(NPB, PBQ, KAUG, T).transpose(0, 2, 1, 3).reshape(
                NPB, KAUG, PBQ * T))
        qt = np.ascontiguousarray(q_in[:, qs, :].reshape(TOKQ, F).T).astype(bf)
        m = dict(shared)
        m["posk_aug"] = pa
        m["q_t"] = qt
        per_core.append(m)
    flags = dict(bv_nz=bool(np.any(bv)), bo_nz=bool(np.any(bo)))
    return per_core, flags


_PROGRAM_CACHE = {}


def kernel(**inputs):
    per_core, flags = make_core_inputs(inputs, CFG)
    key = (tuple(sorted(CFG.items())), tuple(sorted(flags.items())))
    if key not in _PROGRAM_CACHE:
        _PROGRAM_CACHE[key] = build_program(CFG, **flags)
    nc = _PROGRAM_CACHE[key]
    res = run_bass_kernel_spmd(nc, per_core, core_ids=list(range(NCORES)))
    outs = [np.asarray(res.results[c]["out"]).reshape(TQ, B, F).transpose(1, 0, 2)
            for c in range(NCORES)]
    return np.concatenate(outs, axis=1).astype(np.float32)


# revision 34
# speedup vs baseline: 1.0304x; 1.0304x over previous
"""Cross multi-headed attention with relative position bias, sharded over 8 trn2 cores.

Sharding: query positions (T1=1024) split 128/core. pos_k (the 256MB tensor) is
indexed by query position, so each core streams a disjoint 1/8 slice exactly
once. K/V are computed from the full x on every core (cheap); outputs are
disjoint q-slices gathered on host.

Layout tricks:
  - LayerNorm is folded on the host: xn^T ships pre-packed in bf16, so the
    device does no stats/affine/transpose work at all.
  - pos_k is host-packed to (qgroup-of-4, dk+B, 4*T2) so it streams as the
    matmul moving operand (contraction dim dk on partitions) in 590KB DMAs.
  - The mask penalty is folded into the Bmat matmul as 8 extra contraction
    rows (one-hot per batch x -1e4 penalty): masking costs zero vector ops and
    masked attn weights are exactly 0 after exp underflow.
  - Bmat comes out of its matmul in per-q tiles (32 bh-rows x T2); an
    SBUF->SBUF DMA row-scatter regroups it into a resident per-(b,h) tensor
    (128q x T2) -- no DRAM roundtrip.  An identity-matmul accumulates it
    straight onto the QK^T scores in PSUM.
  - Softmax denominators come from the exp activation's accum_out for free;
    no max-subtraction needed since scores are bounded (~N(0,2) after scale).
"""

import numpy as np
import ml_dtypes
from contextlib import ExitStack

import concourse.bass as bass
import concourse.tile as tile
import concourse.mybir as mybir
from concourse import bacc
from concourse.bass_utils import run_bass_kernel_spmd
from concourse.masks import make_identity

F32 = mybir.dt.float32
F32R = mybir.dt.float32r
BF16 = mybir.dt.bfloat16

B, T, F, H, DK = 8, 1024, 256, 4, 64
NCORES = 8
TQ = T // NCORES          # query rows per core (128)
TOK = B * T               # tokens for K/V (8192)
TOKQ = B * TQ             # query tokens per core (1024)
NBH = B * H               # 32 (b,h) pairs
KAUG = DK + B             # bmat contraction rows: 64 qk + 8 mask-penalty
GQ = 4                    # q rows per bmat PSUM group
NQU = TQ // GQ            # 32 bmat groups (4-q units)
PBQ = 4                   # q rows per posk DMA block (1 group)
NPB = TQ // PBQ           # 16 posk blocks
NWB = TQ // 16            # 8 bmat write blocks (16 q each)
EPS = 1e-5
PEN = 1e4                 # mask penalty (score -= PEN/8 => exp underflows to 0)

CFG = dict()

AF = mybir.ActivationFunctionType
ALU = mybir.AluOpType


def build_program(cfg=CFG, bv_nz=False, bo_nz=False):
    nc = bacc.Bacc("TRN2", target_bir_lowering=False, debug=False)

    # ---- DRAM I/O ----
    # xn^T in token-halves: [tokhalf, f//128, f%128, tok%4096]
    xnt_d = nc.dram_tensor("xnt", [2, 2, 128, TOK // 2], BF16, kind="ExternalInput")
    qt_d = nc.dram_tensor("q_t", [F, TOKQ], BF16, kind="ExternalInput")
    # pos_k packed 8 q per block: [pb, k, s*T + t] (16KB contiguous rows)
    posk_d = nc.dram_tensor("posk_aug", [NPB, KAUG, PBQ * T], BF16,
                            kind="ExternalInput")
    # permuted identity for the bmat accumulate: P[r, q] = 1 iff q = perm(r)
    idp_d = nc.dram_tensor("idperm", [128, 128], BF16, kind="ExternalInput")
    oneh_d = nc.dram_tensor("onehot", [B, TQ * NBH], BF16, kind="ExternalInput")
    wq_d = nc.dram_tensor("wq", [F, F], BF16, kind="ExternalInput")
    wk_d = nc.dram_tensor("wk2", [F, F], BF16, kind="ExternalInput")
    wv_d = nc.dram_tensor("wv2", [F, F], BF16, kind="ExternalInput")
    wo_d = nc.dram_tensor("wo", [F, F], F32R, kind="ExternalInput")
    bq_d = nc.dram_tensor("bq_cols", [128, 2], F32, kind="ExternalInput")
    bk_d = nc.dram_tensor("bk2_cols", [128, 2], F32, kind="ExternalInput")
    bv_d = nc.dram_tensor("bv2_row", [1, F], BF16, kind="ExternalInput")
    bo_d = nc.dram_tensor("bo_row", [1, F], F32R, kind="ExternalInput")
    ones_d = nc.dram_tensor("ones_row", [1, 128], F32R, kind="ExternalInput")
    # out[q, b*F + f]; host reshapes to (B, TQ, F)
    out_d = nc.dram_tensor("out", [TQ, B * F], F32, kind="ExternalOutput")

    with tile.TileContext(nc) as tc, ExitStack() as ctx:
        consts = ctx.enter_context(tc.tile_pool(name="consts", bufs=1))
        persist = ctx.enter_context(tc.tile_pool(name="persist", bufs=1))

        # ---- constants ----
        id_bf = consts.tile([128, 128], BF16)
        make_identity(nc, id_bf)
        idp_sb = consts.tile([128, 128], BF16)
        nc.gpsimd.dma_start(idp_sb[:], idp_d[:])
        wq_sb = consts.tile([128, 2 * F], BF16)   # [f%128, (f//128)*F + n]
        wk_sb = consts.tile([128, 2 * F], BF16)
        wv_sb = consts.tile([128, 2 * F], BF16)
        wo_sb = consts.tile([64, 4 * F], F32R)    # [hd%64, (hd//64)*F + n]
        for w_sb, w_d in ((wq_sb, wq_d), (wk_sb, wk_d), (wv_sb, wv_d)):
            for fc in range(2):
                nc.gpsimd.dma_start(w_sb[:, fc * F:(fc + 1) * F],
                                    w_d[fc * 128:(fc + 1) * 128, :])
        for ci in range(4):
            nc.gpsimd.dma_start(wo_sb[:, ci * F:(ci + 1) * F],
                                wo_d[ci * 64:(ci + 1) * 64, :])
        bq_sb = consts.tile([128, 2], F32)
        nc.gpsimd.dma_start(bq_sb[:], bq_d[:])
        bk_sb = consts.tile([128, 2], F32)
        nc.gpsimd.dma_start(bk_sb[:], bk_d[:])
        if bv_nz:
            bv_sb = consts.tile([1, F], BF16)
            nc.gpsimd.dma_start(bv_sb[:], bv_d[:])
            ones_bf = consts.tile([1, 128], BF16)
            nc.gpsimd.memset(ones_bf[:], 1.0)
        if bo_nz:
            bo_sb = consts.tile([1, F], F32R)
            nc.gpsimd.dma_start(bo_sb[:], bo_d[:])
            ones_r = consts.tile([1, 128], F32R)
            nc.gpsimd.dma_start(ones_r[:], ones_d[:])

        # ---- persistent activations ----
        kt_sb = persist.tile([128, 2 * TOK], BF16)     # K^T: [n%128, (n//128)*TOK+tok]
        v_sb = persist.tile([128, (TOK // 128) * F], BF16)  # V: [tok%128,(tok//128)*F+hd]
        qq_sb = persist.tile([KAUG, TQ * NBH], BF16)   # [d|b', b*H*TQ + h*TQ + q]
        qq_dup = persist.tile([128, TQ * NBH], BF16)   # rows 64:128 = qq_sb[0:64]
        nc.gpsimd.dma_start(qq_sb[DK:KAUG, :], oneh_d[:])
        dram = ctx.enter_context(tc.tile_pool(name="dram", bufs=1, space="DRAM"))
        # row r = g4*16 + j*4 + qgi holds q = g4*16 + qgi*4 + j (idperm undoes)
        bmat_dram = dram.tile([NBH, NWB, GQ, 4, T], BF16)

        # ================= Phase B: Q^T projection -> qq_sb =================
        with tc.tile_pool(name="qstage", bufs=1) as qstage, \
             tc.tile_pool(name="ps_q", bufs=2, space="PSUM") as ps_q:
            qt_tiles = []
            for fc in range(2):
                qt = qstage.tile([128, TOKQ], BF16, tag=f"qt{fc}")
                nc.sync.dma_start(qt[:], qt_d[fc * 128:(fc + 1) * 128, :])
                qt_tiles.append(qt)
            qv = qq_sb[0:DK].rearrange("p (b h q) -> p b q h", h=H, q=TQ)
            for h in range(H):
                qp = ps_q.tile([DK, TOKQ], F32, tag="qp")
                for half in range(2):
                    for fc in range(2):
                        nc.tensor.matmul(
                            qp[:, half * 512:(half + 1) * 512],
                            wq_sb[:, fc * F + h * DK: fc * F + (h + 1) * DK],
                            qt_tiles[fc][:, half * 512:(half + 1) * 512],
                            start=(fc == 0), stop=(fc == 1))
                src = qp.rearrange("p (b q) -> p b q", b=B)
                bias = bq_sb[64 * (h % 2):64 * (h % 2) + 64, h // 2: h // 2 + 1]
                nc.scalar.activation(qv[:, :, :, h], src, AF.Identity, bias=bias)
            nc.sync.dma_start(qq_dup[64:128, :], qq_sb[0:DK, :])

        # ===== Phases C+D interleaved: K^T/V proj, Bmat stream + regroup =====
        with tc.tile_pool(name="xstage", bufs=1) as xstage, \
             tc.tile_pool(name="poskst", bufs=5) as poskst, \
             tc.tile_pool(name="bmstage", bufs=2) as bmstage, \
             tc.tile_pool(name="ps_bm", bufs=2, space="PSUM") as ps_bm, \
             tc.tile_pool(name="ps_kv", bufs=3, space="PSUM") as ps_kv:

            # full xn^T resident: [f%128, (L*2 + f//128)*4096 + tl]
            xnt = xstage.tile([128, 2 * TOK], BF16, tag="xnt")
            for L in range(2):
                for fc in range(2):
                    nc.scalar.dma_start(
                        xnt[:, (L * 2 + fc) * (TOK // 2):
                            (L * 2 + fc + 1) * (TOK // 2)],
                        xnt_d[L, fc])
            qq_v = qq_sb.rearrange("p (b h q) -> p b h q", h=H, q=TQ)

            def kv_chunk(s):
                # K^T and V for token chunk s (512 tokens)
                L, sl = s // 8, s % 8
                tok0 = s * 512
                for ncI in range(2):
                    kp = ps_kv.tile([128, 512], F32, tag="kv")
                    for fc in range(2):
                        nc.tensor.matmul(
                            kp[:],
                            wk_sb[:, fc * F + ncI * 128: fc * F + (ncI + 1) * 128],
                            xnt[:, (L * 2 + fc) * (TOK // 2) + sl * 512:
                                (L * 2 + fc) * (TOK // 2) + (sl + 1) * 512],
                            start=(fc == 0), stop=(fc == 1))
                    nc.vector.tensor_scalar(
                        out=kt_sb[:, ncI * TOK + tok0: ncI * TOK + tok0 + 512],
                        in0=kp[:], scalar1=bk_sb[:, ncI:ncI + 1], scalar2=None,
                        op0=ALU.add)
                for gp in range(2):  # 2 token-tiles per V psum
                    vp = ps_kv.tile([128, 2 * F], F32, tag="kv")
                    for gi in range(2):
                        g = sl * 4 + gp * 2 + gi
                        for fc in range(2):
                            nc.tensor.matmul(
                                vp[:, gi * F:(gi + 1) * F],
                                xnt[:, (L * 2 + fc) * (TOK // 2) + g * 128:
                                    (L * 2 + fc) * (TOK // 2) + (g + 1) * 128],
                                wv_sb[:, fc * F:(fc + 1) * F],
                                start=(fc == 0), stop=(fc == 1 and not bv_nz))
                        if bv_nz:
                            nc.tensor.matmul(vp[:, gi * F:(gi + 1) * F],
                                             ones_bf[:], bv_sb[:],
                                             start=False, stop=True)
                    g0 = s * 4 + gp * 2
                    nc.vector.tensor_copy(v_sb[:, g0 * F:(g0 + 2) * F], vp[:])

            pk = None
            bs_wide = None
            for qu in range(NQU):
                pk = poskst.tile([KAUG, PBQ * T], BF16, tag="pk")
                nc.sync.dma_start(pk[:], posk_d[qu])
                if qu % 4 == 0:
                    bs_wide = bmstage.tile([128, 4 * T], BF16, tag="bsw")
                bp = ps_bm.tile([128, T], F32, tag="bp")
                for j in range(GQ):
                    q = qu * GQ + j
                    lhs = qq_v[:, :, :, q]
                    pc = j * T
                    for half in range(2):
                        nc.tensor.matmul(
                            bp[32 * j:32 * (j + 1), half * 512:(half + 1) * 512],
                            lhs,
                            pk[:, pc + half * 512: pc + (half + 1) * 512],
                            start=True, stop=True, tile_position=(0, 32 * j))
                wsub = qu % 4
                nc.scalar.activation(bs_wide[:, wsub * T:(wsub + 1) * T],
                                     bp[:], AF.Copy)
                if wsub == 3:
                    # write block: row (j, bh) -> dram[bh, g4, j, qgi, t] (8KB)
                    dst = bmat_dram[:, qu // 4].rearrange(
                        "bh j qgi t -> j bh qgi t")
                    nc.scalar.dma_start(dst, bs_wide[:])
                if qu % 2 == 1:
                    kv_chunk(qu // 2)

        # ================= Phase E: attention per (b, h) =================
        with tc.tile_pool(name="attst", bufs=4) as attst, \
             tc.tile_pool(name="ps_s", bufs=2, space="PSUM") as ps_s, \
             tc.tile_pool(name="ps_at", bufs=2, space="PSUM") as ps_at, \
             tc.tile_pool(name="ps_small", bufs=1, space="PSUM") as ps_small, \
             tc.tile_pool(name="avout", bufs=2) as avout, \
             tc.tile_pool(name="obuf", bufs=1) as obuf:
            qa0 = qq_sb[0:DK].rearrange("p (c q) -> p c q", c=NBH)
            qa1 = qq_dup[64:128].rearrange("p (c q) -> p c q", c=NBH)
            ob_all = obuf.tile([128, B * F], F32)
            for b in range(B):
                avt = avout.tile([64, 4 * TQ], F32R, tag="avt")  # [hd%64,(hd//64)*TQ+q]
                for h in range(H):
                    bh = H * b + h
                    lhsq = (qa0 if h % 2 == 0 else qa1)[:, bh, :]
                    bm = attst.tile([128, T], BF16, tag="bm")
                    nc.sync.dma_start(bm[:], bmat_dram[bh])
                    sp = ps_s.tile([128, T], F32, tag="sp")
                    for half in range(2):
                        nc.tensor.matmul(
                            sp[:, half * 512:(half + 1) * 512], lhsq,
                            kt_sb[(h % 2) * 64:(h % 2) * 64 + 64,
                                  (h // 2) * TOK + b * T + half * 512:
                                  (h // 2) * TOK + b * T + (half + 1) * 512],
                            start=True, stop=False)
                        nc.tensor.matmul(
                            sp[:, half * 512:(half + 1) * 512], idp_sb[:],
                            bm[:, half * 512:(half + 1) * 512],
                            start=False, stop=True)
                    attn = attst.tile([128, T], BF16, tag="attn")
                    den = attst.tile([128, 1], F32, tag="den")
                    nc.scalar.activation(attn[:], sp[:], AF.Exp,
                                         scale=1.0 / np.sqrt(DK), accum_out=den[:])
                    rn = attst.tile([128, 1], F32, tag="rn")
                    nc.vector.reciprocal(rn[:], den[:])
                    # normalize in [q, t] layout (per-partition scale) on DVE
                    ats = attst.tile([128, T], BF16, tag="ats")
                    nc.vector.tensor_scalar(out=ats[:], in0=attn[:], scalar1=rn[:],
                                            scalar2=None, op0=ALU.mult)
                    att = attst.tile([128, T], BF16, tag="att")  # attn^T
                    tp2 = ps_at.tile([128, T], BF16, tag="tp2")
                    for c in range(8):
                        nc.tensor.transpose(tp2[:, c * 128:(c + 1) * 128],
                                            ats[:, c * 128:(c + 1) * 128], id_bf[:])
                    nc.vector.tensor_copy(att[:], tp2[:])
                    # AV with v stationary: out = av^T [d, q], feeds out-proj direct
                    avp = ps_small.tile([DK, TQ], F32, tag="avp")
                    for c in range(8):
                        nc.tensor.matmul(
                            avp[:],
                            v_sb[:, (b * 8 + c) * F + h * DK:
                                 (b * 8 + c) * F + (h + 1) * DK],
                            att[:, c * 128:(c + 1) * 128],
                            start=(c == 0), stop=(c == 7))
                    nc.vector.tensor_copy(avt[:, h * TQ:(h + 1) * TQ], avp[:])
                op = ps_small.tile([128, F], F32, tag="op")
                for ci in range(4):
                    nc.tensor.matmul(
                        op[:], avt[:, ci * TQ:(ci + 1) * TQ],
                        wo_sb[:, ci * F:(ci + 1) * F],
                        start=(ci == 0), stop=(ci == 3 and not bo_nz))
                if bo_nz:
                    nc.tensor.matmul(op[:], ones_r[:], bo_sb[:],
                                     start=False, stop=True)
                nc.vector.tensor_copy(ob_all[:, b * F:(b + 1) * F], op[:])
            nc.scalar.dma_start(out_d[:], ob_all[:])

    nc._dbg_names = {
        "kt": kt_sb.tensor.name, "v": v_sb.tensor.name,
        "qq": qq_sb.tensor.name, "bmat": bmat_dram.tensor.name,
    }
    nc.compile()
    return nc


def make_core_inputs(inputs, cfg=CFG):
    """Host-side sharding/layout. Returns (per_core_maps, bias_flags)."""
    x = np.asarray(inputs["x"], np.float32)
    q_in = np.asarray(inputs["q_in"], np.float32)
    pos_k = np.asarray(inputs["pos_k"], np.float32)
    mask = np.asarray(inputs["mask"])
    ln_g = np.asarray(inputs["ln_g"], np.float32)
    ln_b = np.asarray(inputs["ln_b"], np.float32)
    Wq, bq = np.asarray(inputs["Wq"], np.float32), np.asarray(inputs["bq"], np.float32)
    Wk, bk = np.asarray(inputs["Wk"], np.float32), np.asarray(inputs["bk"], np.float32)
    Wv, bv = np.asarray(inputs["Wv"], np.float32), np.asarray(inputs["bv"], np.float32)
    Wo, bo = np.asarray(inputs["Wo"], np.float32), np.asarray(inputs["bo"], np.float32)

    bf = ml_dtypes.bfloat16

    # LayerNorm on host (f32), then pack xn^T
    mu = x.mean(-1, keepdims=True)
    xc = x - mu
    var = (xc * xc).mean(-1, keepdims=True)
    xn = xc / np.sqrt(var + EPS) * ln_g + ln_b
    # xnt_d[L, fc, p, tl] = xn^T[fc*128+p, L*4096+tl]
    xnt = np.ascontiguousarray(
        xn.reshape(TOK, F).T.reshape(2, 128, 2, TOK // 2).transpose(2, 0, 1, 3)
    ).astype(bf)

    shared = {
        "xnt": xnt,
        "wq": Wq.astype(bf),
        "wk2": Wk.astype(bf),
        "wv2": Wv.astype(bf),
        "wo": Wo.astype(np.float32),
        "bq_cols": np.ascontiguousarray(bq.reshape(2, 128).T).astype(np.float32),
        "bk2_cols": np.ascontiguousarray(bk.reshape(2, 128).T).astype(np.float32),
        "bv2_row": bv.reshape(1, F).astype(bf),
        "bo_row": bo.reshape(1, F).astype(np.float32),
        "ones_row": np.ones((1, 128), np.float32),
    }
    # onehot penalty rows: qq_aug[DK+b', b*H*TQ + h*TQ + q] = -PEN iff b'==b
    oh = np.zeros((B, B, H * TQ), np.float32)
    for bb in range(B):
        oh[bb, bb, :] = -PEN
    shared["onehot"] = np.ascontiguousarray(oh.reshape(B, NBH * TQ)).astype(bf)
    # permuted identity: dram row r = g4*16 + j*4 + qgi holds q = g4*16 + qgi*4 + j
    idp = np.zeros((128, 128), np.float32)
    r = np.arange(128)
    qperm = (r // 16) * 16 + (r % 4) * 4 + (r // 4) % 4
    idp[r, qperm] = 1.0
    shared["idperm"] = idp.astype(bf)

    per_core = []
    for c in range(NCORES):
        qs = slice(c * TQ, (c + 1) * TQ)
        pa = np.empty((TQ, KAUG, T), bf)
        pa[:, :DK, :] = pos_k[qs].transpose(0, 2, 1).astype(bf)
        pa[:, DK:, :] = (1.0 - mask[:, qs, :].astype(np.float32)
                         ).transpose(1, 0, 2).astype(bf)
        # pack PBQ q per block: [pb, k, s*T + t]
        pa = np.ascontiguousarray(
            pa.reshape(NPB, PBQ, KAUG, T).transpose(0, 2, 1, 3).reshape(
                NPB, KAUG, PBQ * T))
        qt = np.ascontiguousarray(q_in[:, qs, :].reshape(TOKQ, F).T).astype(bf)
        m = dict(shared)
        m["posk_aug"] = pa
        m["q_t"] = qt
        per_core.append(m)
    flags = dict(bv_nz=bool(np.any(bv)), bo_nz=bool(np.any(bo)))
    return per_core, flags


_PROGRAM_CACHE = {}


def kernel(**inputs):
    per_core, flags = make_core_inputs(inputs, CFG)
    key = (tuple(sorted(CFG.items())), tuple(sorted(flags.items())))
    if key not in _PROGRAM_CACHE:
        _PROGRAM_CACHE[key] = build_program(CFG, **flags)
    nc = _PROGRAM_CACHE[key]
    res = run_bass_kernel_spmd(nc, per_core, core_ids=list(range(NCORES)))
    outs = [np.asarray(res.results[c]["out"]).reshape(TQ, B, F).transpose(1, 0, 2)
            for c in range(NCORES)]
    return np.concatenate(outs, axis=1).astype(np.float32)
